# revision 1
# baseline (speedup 1.0000x reference)
"""Trainium2 Bass kernel for nn_Conv3DNorm (modulated conv3d + demod + lrelu + clamp).

Reference math (styles == ones):
    dcoef[cout] = rsqrt(sum_{cin,kd,kh,kw} weight^2 + 1e-8)
    y = conv3d(x, weight * dcoef, pad=1)            # per-sample, stride 1
    y = leaky_relu(y + bias, 0.2) * sqrt(2)
    y = clip(y, -256, 256)

Sharding: data-parallel over batch. Core i processes sample i (B=8 == n_cores).
Weight/bias replicated. Everything on device except input layout prep:
  - weight pre-transposed on host to [cin, tap, cout] (matmul lhsT layout)
  - conv is computed as 27 accumulated matmuls (one per kernel tap) over a
    zero-padded (H,W)-padded input volume resident in SBUF; depth taps that
    fall outside the volume are skipped (implicit D padding).
  - matmul runs in float32r (TF32-like, 1 cycle/row at N>=512) by default.
"""

import os
import sys

for _p in (
    "/root/.axon_site",
    "/root/.axon_site/_ro/trn_rl_repo",
    "/root/.axon_site/_ro/pypackages",
):
    if os.path.isdir(_p) and _p not in sys.path:
        sys.path.insert(0, _p)

import numpy as np

import concourse.bass as bass  # noqa: F401
import concourse.mybir as mybir
import concourse.tile as tile
from concourse import bacc
from concourse.bass_utils import run_bass_kernel_spmd

# Problem constants (hardcoded per contract).
B = 8
CIN = 128
COUT = 128
D = H = W = 32
K = 3
NTAPS = K * K * K  # 27
HP = H + 2  # 34
WP = W + 2  # 34
NCHUNK = 64  # output chunks of 512 spatial positions: (d, half-of-H)
EPS = 1e-8
S1 = float(np.sqrt(2.0))  # ACT_GAIN * GAIN
CLAMP = 256.0
ALPHA = 0.2

# matmul dtype: "f32r" (TF32-like), "bf16", or "f32" (exact, 4x slower)
MM_MODE = os.environ.get("CONV_MM_MODE", "f32r")

LAST_RESULTS = None  # BassKernelResults of the most recent run (for test.py)

_CACHED = {}


def _build_nc(mode: str):
    dt = mybir.dt
    # x / w live in the matmul dtype end-to-end (f32r is a bit-identical
    # reinterpretation of fp32 that the PE runs at 1 cycle/row).
    io_dt = {"f32r": dt.float32r, "bf16": dt.bfloat16, "f32": dt.float32}[mode]

    nc = bacc.Bacc("TRN2")
    x_d = nc.dram_tensor("x", [CIN, D, H, W], io_dt, kind="ExternalInput")
    w_d = nc.dram_tensor("w", [CIN, NTAPS, COUT], io_dt, kind="ExternalInput")
    b_d = nc.dram_tensor("bias", [COUT, 1], dt.float32, kind="ExternalInput")
    y_d = nc.dram_tensor("y", [COUT, NCHUNK, 512], dt.float32, kind="ExternalOutput")

    def asf32(ap):
        return ap.bitcast(dt.float32) if mode == "f32r" else ap

    with tile.TileContext(nc) as tc:
        with (
            tc.tile_pool(name="big", bufs=1) as big,
            tc.tile_pool(name="small", bufs=1) as small,
            tc.tile_pool(name="sq", bufs=2) as sqp,
            tc.tile_pool(name="epiv", bufs=4) as vp,
            tc.tile_pool(name="epio", bufs=4) as op,
        ):
            # ---- weights + bias in SBUF ----
            w_sb = big.tile([CIN, NTAPS, COUT], io_dt)
            nc.sync.dma_start(w_sb[:], w_d[:])
            bias_sb = small.tile([COUT, 1], dt.float32)
            nc.sync.dma_start(bias_sb[:], b_d[:])

            # ---- padded input volume in SBUF: [cin, d, h+2, w+2] ----
            xpad = big.tile([CIN, D, HP, WP], io_dt)
            # zero the (H,W) halo once (bitcast: memset lacks f32r support).
            # These go first on DVE so they don't gate the first conv matmul.
            nc.vector.memset(asf32(xpad[:, :, 0, :]), 0.0)
            nc.vector.memset(asf32(xpad[:, :, HP - 1, :]), 0.0)
            nc.vector.memset(asf32(xpad[:, :, 1 : HP - 1, 0]), 0.0)
            nc.vector.memset(asf32(xpad[:, :, 1 : HP - 1, WP - 1]), 0.0)
            # interior: one DMA per depth slice, on the SWDGE queue so they
            # run in parallel with the w/bias DMAs on the HWDGE queue
            for d in range(D):
                nc.gpsimd.dma_start(
                    xpad[:, d, 1 : HP - 1, 1 : WP - 1], x_d[:, d, :, :]
                )

            # ---- demodulation coefficients (emitted after chunk 0's matmuls
            # so the 53-op DVE square-accumulate chain doesn't delay the first
            # conv matmul; its one PE matmul slots between chunks 0 and 1) ----
            scal = {}

            def emit_dcoef(dcps):
                # acc[cin,cout] = sum_tap w^2 (DVE), then one matmul with ones
                # reduces over cin: ps_dc[cout,1] = acc.T @ ones.
                ones = small.tile([CIN, 1], dt.float32)
                nc.vector.memset(ones[:], 1.0)
                eps_t = small.tile([COUT, 1], dt.float32)
                nc.vector.memset(eps_t[:], EPS)
                acc = small.tile([CIN, COUT], dt.float32)
                nc.vector.tensor_mul(
                    acc[:], asf32(w_sb[:, 0, :]), asf32(w_sb[:, 0, :])
                )
                for t in range(1, NTAPS):
                    sq = sqp.tile([CIN, COUT], dt.float32)
                    nc.vector.tensor_mul(
                        sq[:], asf32(w_sb[:, t, :]), asf32(w_sb[:, t, :])
                    )
                    nc.vector.tensor_add(acc[:], acc[:], sq[:])
                ps_dc = dcps.tile([COUT, 1], dt.float32)
                nc.tensor.matmul(ps_dc[:], acc[:], ones[:], start=True, stop=True)
                # dscale = sqrt(2) / sqrt(sums + eps)
                rsq = small.tile([COUT, 1], dt.float32)
                nc.scalar.activation(
                    rsq[:], ps_dc[:], mybir.ActivationFunctionType.Sqrt, bias=eps_t[:]
                )
                rec = small.tile([COUT, 1], dt.float32)
                nc.vector.reciprocal(rec[:], rsq[:])
                # epilogue computes v = relu(4*a2) + a2 with
                # a2 = 0.2*sqrt2*(psum*dcoef+bias)
                # == sqrt2 * leaky_relu(psum*dcoef + bias, 0.2)
                dscale2 = small.tile([COUT, 1], dt.float32)
                nc.scalar.mul(dscale2[:], rec[:], ALPHA * S1)
                bias2 = small.tile([COUT, 1], dt.float32)
                nc.scalar.mul(bias2[:], bias_sb[:], ALPHA * S1)
                scal["dscale2"] = dscale2
                scal["bias2"] = bias2

            # ---- main conv loop (chunk-major: each chunk's 27 matmuls are
            # consecutive, so chunk completions stagger by ~6.6us and the
            # epilogues overlap the matmul stream instead of bursting at the
            # end; weight reloads are free — LDWEIGHTS hides behind matmuls) ----
            with (
                tc.tile_pool(name="ps", bufs=7, space="PSUM") as psp,
                tc.tile_pool(name="dcps", bufs=1, space="PSUM") as dcps,
            ):
                for c in range(NCHUNK):
                    d, h0 = c // 2, (c % 2) * 16
                    ps = psp.tile([COUT, 512], dt.float32, name=f"ps_{c}", tag="ps")
                    valid = [t for t in range(NTAPS) if 0 <= d + t // 9 - 1 < D]
                    for t in valid:
                        kd, kh, kw = t // 9, (t // 3) % 3, t % 3
                        rhs = xpad[:, d + kd - 1, h0 + kh : h0 + kh + 16, kw : kw + 32]
                        nc.tensor.matmul(
                            ps[:],
                            w_sb[:, t, :],
                            rhs,
                            start=(t == valid[0]),
                            stop=(t == valid[-1]),
                        )
                    if c == 0:
                        emit_dcoef(dcps)
                    # epilogue: sqrt2*lrelu(psum*dcoef + bias, 0.2) then clamp
                    a2 = vp.tile([COUT, 512], dt.float32)
                    nc.vector.tensor_scalar(
                        out=a2[:],
                        in0=ps[:],
                        scalar1=scal["dscale2"][:],
                        scalar2=scal["bias2"][:],
                        op0=mybir.AluOpType.mult,
                        op1=mybir.AluOpType.add,
                    )
                    r1 = vp.tile([COUT, 512], dt.float32, name=f"r1_{c}", tag="r1")
                    nc.scalar.activation(
                        r1[:],
                        a2[:],
                        mybir.ActivationFunctionType.Relu,
                        scale=1.0 / ALPHA - 1.0,
                    )
                    o = op.tile([COUT, 512], dt.float32)
                    nc.vector.scalar_tensor_tensor(
                        out=o[:],
                        in0=r1[:],
                        scalar=1.0,
                        in1=a2[:],
                        op0=mybir.AluOpType.mult,
                        op1=mybir.AluOpType.add,
                    )
                    oc = op.tile([COUT, 512], dt.float32, name=f"oc_{c}", tag="oc")
                    nc.vector.tensor_scalar(
                        out=oc[:],
                        in0=o[:],
                        scalar1=-CLAMP,
                        scalar2=CLAMP,
                        op0=mybir.AluOpType.max,
                        op1=mybir.AluOpType.min,
                    )
                    nc.sync.dma_start(y_d[:, c, :], oc[:])
    nc.compile()
    return nc


def _get_nc(mode: str):
    if mode not in _CACHED:
        _CACHED[mode] = _build_nc(mode)
    return _CACHED[mode]


def kernel(x: np.ndarray, weight: np.ndarray, bias: np.ndarray) -> np.ndarray:
    global LAST_RESULTS
    mode = MM_MODE
    if mode == "bf16":
        import ml_dtypes

        io = ml_dtypes.bfloat16
    else:
        io = np.float32

    x = np.asarray(x)
    weight = np.asarray(weight, dtype=np.float32)
    bias = np.asarray(bias, dtype=np.float32)

    # [cout, cin, kd, kh, kw] -> [cin, (kd kh kw), cout]
    w_prep = np.ascontiguousarray(
        weight.transpose(1, 2, 3, 4, 0).reshape(CIN, NTAPS, COUT).astype(io)
    )
    b_prep = np.ascontiguousarray(bias.reshape(COUT, 1))

    in_maps = [
        {
            "x": np.ascontiguousarray(x[i].astype(io)),
            "w": w_prep,
            "bias": b_prep,
        }
        for i in range(B)
    ]

    nc = _get_nc(mode)
    trace = bool(int(os.environ.get("CONV_TRACE", "0")))
    res = run_bass_kernel_spmd(
        nc,
        in_maps,
        core_ids=list(range(B)),
        trace=trace,
    )
    LAST_RESULTS = res
    out = np.stack(
        [r["y"].reshape(COUT, D, H, W) for r in res.results], axis=0
    ).astype(np.float32)
    return out



# revision 3
# speedup vs baseline: 1.0283x; 1.0283x over previous
"""Trainium2 Bass kernel for nn_Conv3DNorm (modulated conv3d + demod + lrelu + clamp).

Reference math (styles == ones):
    dcoef[cout] = rsqrt(sum_{cin,kd,kh,kw} weight^2 + 1e-8)
    y = conv3d(x, weight * dcoef, pad=1)            # per-sample, stride 1
    y = leaky_relu(y + bias, 0.2) * sqrt(2)
    y = clip(y, -256, 256)

Sharding: data-parallel over batch. Core i processes sample i (B=8 == n_cores).

Design (v3):
  - dcoef folded into the weights on the host (it only depends on `weight`),
    so the device runs a plain conv + lrelu + clamp.
  - conv = 27 accumulated bf16 matmuls per 512-position output chunk over a
    zero-padded (H,W) input volume in SBUF. bf16 halves the PE weight-load
    (LDWEIGHTS) time vs f32r so it hides completely under the 512-row matmul
    stream (issue interval ~219 ns vs ~255 ns for f32r).
  - input path: each x depth-slice DMAs contiguously (128 x 2KB descriptors)
    into a staging ring on the sync HWDGE queue; a DVE copy scatters it into
    the padded layout right after a cheap contiguous full-plane memset (the
    old strided column memsets cost ~4.4 us each on DVE and delayed the
    first matmul).
  - weights are split into 3 tap-range DMAs with the first-needed taps
    (9..17, the kd=1 block used by the d=0 chunks) landing before the x
    slices on the sync queue; the rest follow. bias rides the Act queue.
  - y goes out over the Activation-engine HWDGE queue; the SWDGE path is
    completely unused, avoiding its ~6.5 us drain at teardown.
  - epilogue per chunk (exact lrelu, no act-table dependency):
        u  = psum*sqrt2 + sqrt2*bias      (Identity activation, Act engine)
        o  = max(alpha*u, u)              (DVE scalar_tensor_tensor)
        oc = clip(o, +-256) -> bf16       (DVE tensor_scalar)
    using lrelu(z) = max(z, alpha*z) for 0 < alpha < 1.
  - last chunk split into two 256-wide halves so its epilogue/DMA overlap
    the final matmuls.
"""

import os
import sys

for _p in (
    "/root/.axon_site",
    "/root/.axon_site/_ro/trn_rl_repo",
    "/root/.axon_site/_ro/pypackages",
):
    if os.path.isdir(_p) and _p not in sys.path:
        sys.path.insert(0, _p)

import numpy as np

import concourse.bass as bass  # noqa: F401
import concourse.mybir as mybir
import concourse.tile as tile
from concourse import bacc
from concourse.bass_utils import run_bass_kernel_spmd

# Problem constants (hardcoded per contract).
B = 8
CIN = 128
COUT = 128
D = H = W = 32
K = 3
NTAPS = K * K * K  # 27
HP = H + 2  # 34
WP = W + 2  # 34
NCHUNK = 64  # output chunks of 512 spatial positions: (d, half-of-H)
EPS = 1e-8
S1 = float(np.sqrt(2.0))  # ACT_GAIN * GAIN
CLAMP = 256.0
ALPHA = 0.2

LAST_RESULTS = None  # BassKernelResults of the most recent run (for test.py)

_CACHED = {}


def _valid_taps(d):
    return [t for t in range(NTAPS) if 0 <= d + t // 9 - 1 < D]


def _build_nc():
    dt = mybir.dt
    io_dt = dt.bfloat16

    nc = bacc.Bacc("TRN2")
    x_d = nc.dram_tensor("x", [CIN, D, H, W], io_dt, kind="ExternalInput")
    w_d = nc.dram_tensor("w", [CIN, NTAPS, COUT], io_dt, kind="ExternalInput")
    b_d = nc.dram_tensor("bias", [COUT, 1], dt.float32, kind="ExternalInput")
    y_d = nc.dram_tensor("y", [COUT, NCHUNK, 512], io_dt, kind="ExternalOutput")

    with tile.TileContext(nc) as tc:
        with (
            tc.tile_pool(name="big", bufs=1) as big,
            tc.tile_pool(name="small", bufs=1) as small,
            tc.tile_pool(name="stg", bufs=4) as stp,
            tc.tile_pool(name="epiv", bufs=4) as vp,
            tc.tile_pool(name="epio", bufs=4) as op,
        ):
            w_sb = big.tile([CIN, NTAPS, COUT], io_dt)
            bias_sb = small.tile([COUT, 1], dt.float32)
            xpad = big.tile([CIN, D, HP, WP], io_dt)

            # first-needed weight taps (kd=1 block, used by the d=0 chunks)
            # land on the sync queue ahead of the x slices
            nc.sync.dma_start(w_sb[:, 9:18, :], w_d[:, 9:18, :])

            def load_slice(d):
                # contiguous plane memset (halo zeroing) + contiguous DMA
                # (128 x 2KB descriptors) + DVE scatter into padded layout
                nc.vector.memset(xpad[:, d, :, :], 0.0)
                st = stp.tile([CIN, H, W], io_dt, name=f"st_{d}", tag="st")
                nc.sync.dma_start(st[:], x_d[:, d, :, :])
                nc.vector.tensor_scalar_add(
                    xpad[:, d, 1 : HP - 1, 1 : WP - 1], st[:], 0.0
                )

            load_slice(0)
            load_slice(1)
            nc.sync.dma_start(w_sb[:, 18:27, :], w_d[:, 18:27, :])
            nc.sync.dma_start(w_sb[:, 0:9, :], w_d[:, 0:9, :])
            nc.scalar.dma_start(bias_sb[:], b_d[:])

            def epilogue(ps_ap, y_ap, width, c, half=""):
                # u = sqrt2*psum + sqrt2*bias; o = max(alpha*u, u) = sqrt2*lrelu
                u = vp.tile([COUT, width], dt.float32)
                nc.scalar.activation(
                    u[:],
                    ps_ap,
                    mybir.ActivationFunctionType.Identity,
                    bias=bias_sb[:],
                    scale=S1,
                )
                o = vp.tile([COUT, width], dt.float32, name=f"o_{c}{half}", tag="o")
                nc.vector.scalar_tensor_tensor(
                    out=o[:],
                    in0=u[:],
                    scalar=ALPHA,
                    in1=u[:],
                    op0=mybir.AluOpType.mult,
                    op1=mybir.AluOpType.max,
                )
                oc = op.tile([COUT, width], io_dt, name=f"oc_{c}{half}", tag="oc")
                nc.vector.tensor_scalar(
                    out=oc[:],
                    in0=o[:],
                    scalar1=-CLAMP,
                    scalar2=CLAMP,
                    op0=mybir.AluOpType.max,
                    op1=mybir.AluOpType.min,
                )
                nc.scalar.dma_start(y_ap, oc[:])

            with (
                tc.tile_pool(name="ps", bufs=6, space="PSUM") as psp,
                tc.tile_pool(name="psh", bufs=2, space="PSUM") as psh,
            ):
                for c in range(NCHUNK):
                    d, h0 = c // 2, (c % 2) * 16
                    if c % 2 == 0 and d + 1 < D:
                        # stage the next depth slice one chunk-pair ahead
                        load_slice(d + 1)
                    valid = _valid_taps(d)
                    if c < NCHUNK - 1:
                        ps = psp.tile([COUT, 512], dt.float32, name=f"ps_{c}", tag="ps")
                        for t in valid:
                            kd, kh, kw = t // 9, (t // 3) % 3, t % 3
                            rhs = xpad[
                                :, d + kd - 1, h0 + kh : h0 + kh + 16, kw : kw + 32
                            ]
                            nc.tensor.matmul(
                                ps[:],
                                w_sb[:, t, :],
                                rhs,
                                start=(t == valid[0]),
                                stop=(t == valid[-1]),
                            )
                        epilogue(ps[:], y_d[:, c, :], 512, c)
                    else:
                        # split the last chunk so its epilogue overlaps matmuls
                        for hi in range(2):
                            hh = h0 + 8 * hi
                            ph = psh.tile(
                                [COUT, 256], dt.float32, name=f"ps_{c}_{hi}", tag="psh"
                            )
                            for t in valid:
                                kd, kh, kw = t // 9, (t // 3) % 3, t % 3
                                rhs = xpad[
                                    :, d + kd - 1, hh + kh : hh + kh + 8, kw : kw + 32
                                ]
                                nc.tensor.matmul(
                                    ph[:],
                                    w_sb[:, t, :],
                                    rhs,
                                    start=(t == valid[0]),
                                    stop=(t == valid[-1]),
                                )
                            epilogue(
                                ph[:],
                                y_d[:, c, 256 * hi : 256 * (hi + 1)],
                                256,
                                c,
                                half=f"_{hi}",
                            )
    nc.compile()
    return nc


def _get_nc():
    if "nc" not in _CACHED:
        _CACHED["nc"] = _build_nc()
    return _CACHED["nc"]


def kernel(x: np.ndarray, weight: np.ndarray, bias: np.ndarray) -> np.ndarray:
    global LAST_RESULTS
    import ml_dtypes

    io = ml_dtypes.bfloat16

    x = np.asarray(x)
    weight = np.asarray(weight, dtype=np.float32)
    bias = np.asarray(bias, dtype=np.float32)

    # demodulation coefficients (styles == ones) folded into the weights
    dcoef = 1.0 / np.sqrt(
        np.sum(np.square(weight.astype(np.float64)), axis=(1, 2, 3, 4)) + EPS
    )
    w_fold = weight * dcoef[:, None, None, None, None].astype(np.float32)
    # [cout, cin, kd, kh, kw] -> [cin, (kd kh kw), cout]
    w_prep = np.ascontiguousarray(
        w_fold.transpose(1, 2, 3, 4, 0).reshape(CIN, NTAPS, COUT).astype(io)
    )
    # epilogue computes max(alpha*u, u) with u = sqrt2*psum + sqrt2*bias
    b_prep = np.ascontiguousarray((S1 * bias).reshape(COUT, 1).astype(np.float32))

    in_maps = [
        {
            "x": np.ascontiguousarray(x[i].astype(io)),
            "w": w_prep,
            "bias": b_prep,
        }
        for i in range(B)
    ]

    nc = _get_nc()
    trace = bool(int(os.environ.get("CONV_TRACE", "0")))
    res = run_bass_kernel_spmd(
        nc,
        in_maps,
        core_ids=list(range(B)),
        trace=trace,
    )
    LAST_RESULTS = res
    out = np.stack(
        [r["y"].astype(np.float32).reshape(COUT, D, H, W) for r in res.results],
        axis=0,
    )
    return out


# revision 6
# speedup vs baseline: 1.0302x; 1.0018x over previous
"""Trainium2 Bass kernel for nn_Conv3DNorm (modulated conv3d + demod + lrelu + clamp).

Reference math (styles == ones):
    dcoef[cout] = rsqrt(sum_{cin,kd,kh,kw} weight^2 + 1e-8)
    y = conv3d(x, weight * dcoef, pad=1)            # per-sample, stride 1
    y = leaky_relu(y + bias, 0.2) * sqrt(2)
    y = clip(y, -256, 256)

Sharding: data-parallel over batch. Core i processes sample i (B=8 == n_cores).

Design (v3):
  - dcoef folded into the weights on the host (it only depends on `weight`),
    so the device runs a plain conv + lrelu + clamp.
  - conv = 27 accumulated bf16 matmuls per 512-position output chunk over a
    zero-padded (H,W) input volume in SBUF. bf16 halves the PE weight-load
    (LDWEIGHTS) time vs f32r so it hides completely under the 512-row matmul
    stream (issue interval ~219 ns vs ~255 ns for f32r).
  - input path: each x depth-slice DMAs contiguously (128 x 2KB descriptors)
    into a staging ring on the sync HWDGE queue; a DVE copy scatters it into
    the padded layout right after a cheap contiguous full-plane memset (the
    old strided column memsets cost ~4.4 us each on DVE and delayed the
    first matmul).
  - weights are split into 3 tap-range DMAs with the first-needed taps
    (9..17, the kd=1 block used by the d=0 chunks) landing before the x
    slices on the sync queue; the rest follow. bias rides the Act queue.
  - y goes out over the Activation-engine HWDGE queue; the SWDGE path is
    completely unused, avoiding its ~6.5 us drain at teardown.
  - epilogue per chunk (exact lrelu, no act-table dependency):
        u  = psum*sqrt2 + sqrt2*bias      (Identity activation, Act engine)
        o  = max(alpha*u, u)              (DVE scalar_tensor_tensor)
        oc = clip(o, +-256) -> bf16       (DVE tensor_scalar)
    using lrelu(z) = max(z, alpha*z) for 0 < alpha < 1.
  - last chunk split into two 256-wide halves so its epilogue/DMA overlap
    the final matmuls.
"""

import os
import sys

for _p in (
    "/root/.axon_site",
    "/root/.axon_site/_ro/trn_rl_repo",
    "/root/.axon_site/_ro/pypackages",
):
    if os.path.isdir(_p) and _p not in sys.path:
        sys.path.insert(0, _p)

import numpy as np

import concourse.bass as bass  # noqa: F401
import concourse.mybir as mybir
import concourse.tile as tile
from concourse import bacc
from concourse.bass_utils import run_bass_kernel_spmd

# Problem constants (hardcoded per contract).
B = 8
CIN = 128
COUT = 128
D = H = W = 32
K = 3
NTAPS = K * K * K  # 27
HP = H + 2  # 34
WP = W + 2  # 34
NCHUNK = 64  # output chunks of 512 spatial positions: (d, half-of-H)
EPS = 1e-8
S1 = float(np.sqrt(2.0))  # ACT_GAIN * GAIN
CLAMP = 256.0
ALPHA = 0.2

LAST_RESULTS = None  # BassKernelResults of the most recent run (for test.py)

_CACHED = {}


def _valid_taps(d):
    return [t for t in range(NTAPS) if 0 <= d + t // 9 - 1 < D]


def _build_nc():
    dt = mybir.dt
    io_dt = dt.bfloat16

    nc = bacc.Bacc("TRN2")
    x_d = nc.dram_tensor("x", [CIN, D, H, W], io_dt, kind="ExternalInput")
    w_d = nc.dram_tensor("w", [CIN, NTAPS, COUT], io_dt, kind="ExternalInput")
    b_d = nc.dram_tensor("bias", [COUT, 1], dt.float32, kind="ExternalInput")
    y_d = nc.dram_tensor("y", [COUT, NCHUNK, 512], io_dt, kind="ExternalOutput")

    with tile.TileContext(nc) as tc:
        with (
            tc.tile_pool(name="big", bufs=1) as big,
            tc.tile_pool(name="small", bufs=1) as small,
            tc.tile_pool(name="stg", bufs=4) as stp,
            tc.tile_pool(name="epiv", bufs=4) as vp,
            tc.tile_pool(name="epio", bufs=4) as op,
        ):
            w_sb = big.tile([CIN, NTAPS, COUT], io_dt)
            bias_sb = small.tile([COUT, 1], dt.float32)
            xpad = big.tile([CIN, D, HP, WP], io_dt)

            def load_slice(d):
                # contiguous plane memset (halo zeroing) + contiguous DMA
                # (128 x 2KB descriptors) + DVE scatter into padded layout
                nc.vector.memset(xpad[:, d, :, :], 0.0)
                st = stp.tile([CIN, H, W], io_dt, name=f"st_{d}", tag="st")
                nc.sync.dma_start(st[:], x_d[:, d, :, :])
                nc.vector.tensor_scalar_add(
                    xpad[:, d, 1 : HP - 1, 1 : WP - 1], st[:], 0.0
                )

            # sync-queue order tuned so the first matmul's inputs land first:
            # slice 0, tap 9's weights (the first LDWEIGHTS), slice 1, then
            # the remaining weight taps in consumption order
            load_slice(0)
            nc.sync.dma_start(w_sb[:, 9:10, :], w_d[:, 9:10, :])
            load_slice(1)
            nc.sync.dma_start(w_sb[:, 10:18, :], w_d[:, 10:18, :])
            nc.sync.dma_start(w_sb[:, 18:27, :], w_d[:, 18:27, :])
            nc.sync.dma_start(w_sb[:, 0:9, :], w_d[:, 0:9, :])
            nc.scalar.dma_start(bias_sb[:], b_d[:])

            def epilogue(ps_ap, oc_ap, width, c, half=""):
                # u = sqrt2*psum + sqrt2*bias; o = max(alpha*u, u) = sqrt2*lrelu
                u = vp.tile([COUT, width], dt.float32)
                nc.scalar.activation(
                    u[:],
                    ps_ap,
                    mybir.ActivationFunctionType.Identity,
                    bias=bias_sb[:],
                    scale=S1,
                )
                o = vp.tile([COUT, width], dt.float32, name=f"o_{c}{half}", tag="o")
                nc.vector.scalar_tensor_tensor(
                    out=o[:],
                    in0=u[:],
                    scalar=ALPHA,
                    in1=u[:],
                    op0=mybir.AluOpType.mult,
                    op1=mybir.AluOpType.max,
                )
                nc.vector.tensor_scalar(
                    out=oc_ap,
                    in0=o[:],
                    scalar1=-CLAMP,
                    scalar2=CLAMP,
                    op0=mybir.AluOpType.max,
                    op1=mybir.AluOpType.min,
                )

            with (
                tc.tile_pool(name="ps", bufs=6, space="PSUM") as psp,
                tc.tile_pool(name="psh", bufs=2, space="PSUM") as psh,
            ):
                ocp = None
                for c in range(NCHUNK):
                    d, h0 = c // 2, (c % 2) * 16
                    if c % 2 == 0 and d + 1 < D:
                        # stage the next depth slice one chunk-pair ahead
                        load_slice(d + 1)
                    valid = _valid_taps(d)
                    if c < NCHUNK - 2:
                        # y DMAs are paired (one DMA per two chunks) to halve
                        # the semaphore count the exit barrier has to clear
                        if c % 2 == 0:
                            ocp = op.tile(
                                [COUT, 2, 512], io_dt, name=f"oc_{c}", tag="oc"
                            )
                        ps = psp.tile([COUT, 512], dt.float32, name=f"ps_{c}", tag="ps")
                        for t in valid:
                            kd, kh, kw = t // 9, (t // 3) % 3, t % 3
                            rhs = xpad[
                                :, d + kd - 1, h0 + kh : h0 + kh + 16, kw : kw + 32
                            ]
                            nc.tensor.matmul(
                                ps[:],
                                w_sb[:, t, :],
                                rhs,
                                start=(t == valid[0]),
                                stop=(t == valid[-1]),
                            )
                        epilogue(ps[:], ocp[:, c % 2, :], 512, c)
                        if c % 2 == 1:
                            nc.scalar.dma_start(y_d[:, c - 1 : c + 1, :], ocp[:])
                    elif c == NCHUNK - 2:
                        ps = psp.tile([COUT, 512], dt.float32, name=f"ps_{c}", tag="ps")
                        for t in valid:
                            kd, kh, kw = t // 9, (t // 3) % 3, t % 3
                            rhs = xpad[
                                :, d + kd - 1, h0 + kh : h0 + kh + 16, kw : kw + 32
                            ]
                            nc.tensor.matmul(
                                ps[:],
                                w_sb[:, t, :],
                                rhs,
                                start=(t == valid[0]),
                                stop=(t == valid[-1]),
                            )
                        oc = op.tile([COUT, 512], io_dt, name=f"oc_{c}", tag="oc")
                        epilogue(ps[:], oc[:], 512, c)
                        nc.scalar.dma_start(y_d[:, c, :], oc[:])
                    else:
                        # split the last chunk so its epilogue overlaps matmuls
                        for hi in range(2):
                            hh = h0 + 8 * hi
                            ph = psh.tile(
                                [COUT, 256], dt.float32, name=f"ps_{c}_{hi}", tag="psh"
                            )
                            for t in valid:
                                kd, kh, kw = t // 9, (t // 3) % 3, t % 3
                                rhs = xpad[
                                    :, d + kd - 1, hh + kh : hh + kh + 8, kw : kw + 32
                                ]
                                nc.tensor.matmul(
                                    ph[:],
                                    w_sb[:, t, :],
                                    rhs,
                                    start=(t == valid[0]),
                                    stop=(t == valid[-1]),
                                )
                            oc = op.tile(
                                [COUT, 256], io_dt, name=f"oc_{c}_{hi}", tag="oc"
                            )
                            epilogue(ph[:], oc[:], 256, c, half=f"_{hi}")
                            nc.scalar.dma_start(
                                y_d[:, c, 256 * hi : 256 * (hi + 1)], oc[:]
                            )
    nc.compile()
    return nc


def _get_nc():
    if "nc" not in _CACHED:
        _CACHED["nc"] = _build_nc()
    return _CACHED["nc"]


def kernel(x: np.ndarray, weight: np.ndarray, bias: np.ndarray) -> np.ndarray:
    global LAST_RESULTS
    import ml_dtypes

    io = ml_dtypes.bfloat16

    x = np.asarray(x)
    weight = np.asarray(weight, dtype=np.float32)
    bias = np.asarray(bias, dtype=np.float32)

    # demodulation coefficients (styles == ones) folded into the weights
    dcoef = 1.0 / np.sqrt(
        np.sum(np.square(weight.astype(np.float64)), axis=(1, 2, 3, 4)) + EPS
    )
    w_fold = weight * dcoef[:, None, None, None, None].astype(np.float32)
    # [cout, cin, kd, kh, kw] -> [cin, (kd kh kw), cout]
    w_prep = np.ascontiguousarray(
        w_fold.transpose(1, 2, 3, 4, 0).reshape(CIN, NTAPS, COUT).astype(io)
    )
    # epilogue computes max(alpha*u, u) with u = sqrt2*psum + sqrt2*bias
    b_prep = np.ascontiguousarray((S1 * bias).reshape(COUT, 1).astype(np.float32))

    in_maps = [
        {
            "x": np.ascontiguousarray(x[i].astype(io)),
            "w": w_prep,
            "bias": b_prep,
        }
        for i in range(B)
    ]

    nc = _get_nc()
    trace = bool(int(os.environ.get("CONV_TRACE", "0")))
    res = run_bass_kernel_spmd(
        nc,
        in_maps,
        core_ids=list(range(B)),
        trace=trace,
    )
    LAST_RESULTS = res
    out = np.stack(
        [r["y"].astype(np.float32).reshape(COUT, D, H, W) for r in res.results],
        axis=0,
    )
    return out
